# revision 11
# baseline (speedup 1.0000x reference)
"""Trainium2 Bass kernel for nn_GATv2Model (gnn_message_passing).

Sharding: data-parallel over batch (B=16 -> 2 graphs x 8 NeuronCores), all
weights/constants replicated; per-core compute fully SBUF-resident.

Math (reference):
  h = LN2(relu(LN1(x@in1W.T))@in2W.T)
  per layer: gs = h@Wl.T+bl; gd = h@Wr.T+br       [B,N,H,C]
    z[b,i,j] = gs[b,i] + gd[b,j] + ee[cat[i,j]]
    a = att.leaky_relu(z) = 0.6*(su_i+sv_j+se_cat) + sum_c 0.4*att*|z|
    A = softmax_i(a); o = A.gs; h += relu(LN(o@pW.T + cb-fold))
Factorizations used:
  * cat[i,j] = 3p(i)+q(j) off-diagonal (12 categories): ee enters via
    gs3_q = gs + eet[3p+q] (9 cheap per-orbit adds, not N^2 work).
  * softmax normalizer cancels per-(b,j,h) constants => sv term dropped.
  * unnormalized exp: E = exp(a'), o = (E.[gs|1]) num/den; scores are O(1)
    so no max-subtraction needed.
  * diagonal category (cat[i,i]=9+p): computed separately per node for both
    categories; dE = exp(true)-exp(block) added to num/den.
Score inner loop per (b,J): az[hc,i] = |gs3_q(J) + gd[:,J]| as ONE fused DVE
tensor_scalar (op0=add, op1=abs_max) in bf16; PE contracts az [128,m<=128]
against block-diag 0.4*att [128,8] accumulating S_psum[i,(J,h)]; linear terms
enter PSUM via k=3 (orbit one-hot) and k=8 (head-delta) matmuls; ACT applies
exp while evicting PSUM -> E[i,(h,J)] bf16 which is exactly the layout the
aggregation matmuls (contract over i) consume. No transposes in the loop.
"""

import numpy as np
from contextlib import ExitStack

import concourse.bass as bass
import concourse.tile as tile
from concourse import mybir
from concourse.bass_utils import run_bass_kernel_spmd
from concourse.bass_interp import get_hw_module
from concourse.masks import make_identity

# ---------------------------------------------------------------- constants
N, D, H, C, L, B = 207, 128, 8, 16, 4, 16
NO, SZ, EPS = 3, 69, 1e-5
NCORES = 8
BPC = B // NCORES          # graphs per core
BN = BPC * N               # 414
NP = 208                   # padded N (even free dims)
JG = 64                    # J-group: 64*8 = 512 PSUM f32 cols = 1 bank
F32 = mybir.dt.float32
BF16 = mybir.dt.bfloat16
AOP = mybir.AluOpType
AFT = mybir.ActivationFunctionType

POS = (np.arange(N) // SZ).tolist()
NCHUNKS = [(0, 128), (128, N)]
PBLOCKS = [(p * SZ, min((p + 1) * SZ, N)) for p in range(NO)]
JGROUPS = [(g * JG, min((g + 1) * JG, N)) for g in range((N + JG - 1) // JG)]
CHUNKS = [(b, ci, c0, c1 - c0) for b in range(BPC)
          for ci, (c0, c1) in enumerate(NCHUNKS)]

MAX_WAITS = 1  # this walrus build rejects >1 sync wait per instruction


def _split_excess_waits(nc):
    """Hoist excess per-instruction sync waits onto preceding NoOps."""
    n_fixed = 0
    for f in nc.m.functions:
        for blk in f.blocks:
            if not any(
                i.sync_info and len(i.sync_info.on_wait) > MAX_WAITS
                for i in blk.instructions
            ):
                continue
            new_insts = []
            for inst in blk.instructions:
                si = inst.sync_info
                if si is not None and len(si.on_wait) > MAX_WAITS:
                    waits = list(si.on_wait)
                    extra, keep = waits[:-MAX_WAITS], waits[-MAX_WAITS:]
                    k = 0
                    while extra:
                        chunk, extra = extra[:MAX_WAITS], extra[MAX_WAITS:]
                        new_insts.append(
                            mybir.InstNoOp(
                                name=f"{inst.name}-w{k}",
                                engine=inst.engine,
                                sync_info=mybir.SyncInfo(on_wait=chunk, on_update=[]),
                            )
                        )
                        k += 1
                    inst.sync_info = mybir.SyncInfo(
                        on_wait=keep, on_update=list(si.on_update)
                    )
                    n_fixed += 1
                new_insts.append(inst)
            blk.instructions = new_insts
    return n_fixed


def _bcast_inner(ap_2d, reps):
    """[P, n] AP -> [P, n, reps] AP with step-0 innermost broadcast."""
    dims = list(ap_2d.ap)
    assert len(dims) == 2, dims
    return bass.AP(tensor=ap_2d.tensor, offset=ap_2d.offset,
                   ap=[dims[0], dims[1], [0, reps]])


# ---------------------------------------------------------------- builder
def build_nc(hw=True):
    nc = bass.Bass("TRN2", target_bir_lowering=False, debug=False)

    def din(name, shape, dt=F32):
        return nc.dram_tensor(name, shape, dt, kind="ExternalInput").ap()

    a = {
        "xc": din("xc", [BPC, N, 6]),
        "in1WT": din("in1WT", [6, D]),
        "in2WT": din("in2WT", [D, D]),
        "WT3": din("WT3", [L, 3, D, D]),            # Wl.T, Wr.T, pW.T
        "oWT": din("oWT", [D, 2]),
        "eetT": din("eetT", [L, D, 12]),            # [l, hc, cat]
        "att04": din("att04", [L, D, H], BF16),     # 0.4*att block-diag
        "att06": din("att06", [L, D, H]),           # 0.6*att block-diag
        "se06f": din("se06f", [L, NO, N * H], BF16),
        "sedB": din("sedB", [L, NO, H], BF16),
        "sedT": din("sedT", [L, NO, H], BF16),
        "I8t": din("I8t", [H, JG * H], BF16),
        "P1h": din("P1h", [NO, BN], BF16),
        "bias2": din("bias2", [2, D]),              # in1_b, in2_b
        "ln12": din("ln12", [4, D]),                # g1,b1,g2,b2
        "blbr": din("blbr", [L, 2, D]),
        "pbf": din("pbf", [L, D]),                  # pb + pW@cb
        "lngb": din("lngb", [L, 2, D]),
        "ob2": din("ob2", [2, 1]),
    }
    out_ap = nc.dram_tensor("out", [BPC, N, 2], F32, kind="ExternalOutput").ap()

    with tile.TileContext(nc) as tc:
        with ExitStack() as ctx:
            _body(ctx, tc, a, out_ap)

    if hw:
        nc.m = get_hw_module(nc.m)
        _split_excess_waits(nc)
    return nc


def _body(ctx, tc, a, out_ap):
    nc = tc.nc
    P = 128
    const = ctx.enter_context(tc.tile_pool(name="const", bufs=1))
    state = ctx.enter_context(tc.tile_pool(name="state", bufs=1))
    work = ctx.enter_context(tc.tile_pool(name="work", bufs=4))
    # PSUM: 8 banks total. S0+S1 (1+1) + mmbig (1) + Ops (1) + trF (1) +
    # trB (1) + psd (1) = 7.
    ps_S = ctx.enter_context(tc.tile_pool(name="ps_S", bufs=1, space="PSUM"))
    ps_O = ctx.enter_context(tc.tile_pool(name="ps_O", bufs=1, space="PSUM"))
    ps_mm = ctx.enter_context(tc.tile_pool(name="ps_mm", bufs=1, space="PSUM"))
    ps_tr = ctx.enter_context(tc.tile_pool(name="ps_tr", bufs=1, space="PSUM"))
    ps_sm = ctx.enter_context(tc.tile_pool(name="ps_sm", bufs=1, space="PSUM"))

    # ---------------- constants into SBUF
    def load(name, shape, dt=F32, perm=None, split=None):
        """DMA a[name] (optionally dim-permuted) into a [P0, prod(rest)] tile."""
        t = const.tile(shape, dt, tag=name)
        src = a[name] if perm is None else a[name].rearrange(perm)
        if len(src.shape) > 2:
            names = " ".join(f"d{i}" for i in range(1, len(src.shape)))
            kw = {f"d{i}": src.shape[i] for i in range(1, len(src.shape))}
            dst = t[:].rearrange(f"p ({names}) -> p {names}", **kw)
        else:
            dst = t[:]
        nc.sync.dma_start(out=dst, in_=src)
        return t

    in1WT = load("in1WT", [6, D])
    in2WT = load("in2WT", [D, D])
    WT3 = load("WT3", [D, L * 3 * D], perm="l t i o -> i l t o",
               split="i (l t o) -> i l t o")
    oWT = load("oWT", [D, 2])
    eetT = load("eetT", [D, L * 12], perm="l i c -> i l c",
                split="i (l c) -> i l c")
    att04 = load("att04", [D, L * H], BF16, perm="l i h -> i l h",
                 split="i (l h) -> i l h")
    att06 = load("att06", [D, L * H], perm="l i h -> i l h",
                 split="i (l h) -> i l h")
    se06f = load("se06f", [NO, L * N * H], BF16, perm="l p x -> p l x",
                 split="p (l x) -> p l x")
    sedB = load("sedB", [NO, L * H], BF16, perm="l p h -> p l h",
                split="p (l h) -> p l h")
    sedT = load("sedT", [NO, L * H], BF16, perm="l p h -> p l h",
                split="p (l h) -> p l h")
    I8t = load("I8t", [H, JG * H], BF16)
    P1h = load("P1h", [NO, BN], BF16)
    ob2 = load("ob2", [2, 1])

    # per-partition bias/gain columns, indexed by feature hc (=partition in FM)
    # cols: [in1_b, in2_b, g1, b1, g2, b2, (bl,br)*L, pbf*L, (lng,lnb)*L]
    NBC = 6 + 2 * L + L + 2 * L
    bias_cols = const.tile([D, NBC], F32, tag="bias_cols")
    nc.sync.dma_start(out=bias_cols[:, 0:2], in_=a["bias2"].rearrange("t d -> d t"))
    nc.sync.dma_start(out=bias_cols[:, 2:6], in_=a["ln12"].rearrange("t d -> d t"))
    c_blbr = 6
    nc.sync.dma_start(out=bias_cols[:, c_blbr:c_blbr + 2 * L],
                      in_=a["blbr"].rearrange("l t d -> d (l t)"))
    c_pbf = c_blbr + 2 * L
    nc.sync.dma_start(out=bias_cols[:, c_pbf:c_pbf + L],
                      in_=a["pbf"].rearrange("l d -> d l"))
    c_lngb = c_pbf + L
    nc.sync.dma_start(out=bias_cols[:, c_lngb:c_lngb + 2 * L],
                      in_=a["lngb"].rearrange("l t d -> d (l t)"))

    def bcol(i):
        return bias_cols[:, i:i + 1]

    eps_col = const.tile([P, 1], F32, tag="eps_col")
    nc.vector.memset(eps_col[:], EPS)
    identF = const.tile([P, P], F32, tag="identF")
    make_identity(nc, identF[:])
    identB = const.tile([P, P], BF16, tag="identB")
    nc.vector.tensor_copy(identB[:], identF[:])

    # ---------------- persistent state tiles
    h_fm = state.tile([P, BN], F32, tag="h_fm")
    gs_sb = state.tile([P, BN], F32, tag="gs_sb")
    gd_sb = state.tile([P, BN], F32, tag="gd_sb")
    gs3 = state.tile([P, NO, BPC, NP], BF16, tag="gs3")
    suT = state.tile([H, BN], BF16, tag="suT")
    gs_nm = state.tile([P, 2, BPC, H * 17], BF16, tag="gs_nm")
    E_im = state.tile([P, 2, BPC, H, NP], BF16, tag="E_im")
    dE = state.tile([P, BPC, 2, H], F32, tag="dE")
    o_nm = state.tile([P, 2, BPC, D], F32, tag="o_nm")
    x_sb = state.tile([P, BN], F32, tag="x_sb")
    ofm = state.tile([P, BN], F32, tag="ofm")
    tmp_fm = state.tile([P, BN], F32, tag="tmp_fm")
    x_nm = state.tile([P, 4, D], F32, tag="x_nm")
    xn_nm = state.tile([P, 4, D], F32, tag="xn_nm")
    stats = state.tile([P, 4, 6], F32, tag="stats")
    mv = state.tile([P, 4, 2], F32, tag="mv")
    rstd = state.tile([P, 4, 1], F32, tag="rstd")
    rden = state.tile([P, 2, BPC * H], F32, tag="rden")
    zdiag = state.tile([P, BPC, NP], F32, tag="zdiag")
    azdB = state.tile([P, BPC, NP], BF16, tag="azdB")
    azdT = state.tile([P, BPC, NP], BF16, tag="azdT")
    eBeT = state.tile([P, BPC, 2, 2 * H], F32, tag="eBeT")
    ntmp = state.tile([P, D], F32, tag="ntmp")
    xin = state.tile([P, 4, 6], F32, tag="xin")
    x_fm = state.tile([6, BN], F32, tag="x_fm")
    out_fm = state.tile([2, BN], F32, tag="out_fm")
    out_nm = state.tile([P, 4, 2], F32, tag="out_nm")

    nc.vector.memset(gs3[:], 0.0)
    nc.vector.memset(gs_nm[:], 0.0)
    for ci in range(2):
        for b in range(BPC):
            nc.vector.memset(gs_nm[:, ci, b, 16::17], 1.0)  # den "ones" cols

    # ---------------- helpers
    def mm_big(lhsT, rhs):
        ps = ps_mm.tile([P, BN], F32, tag="mmbig")
        nc.tensor.matmul(ps[:, 0:BN], lhsT, rhs, start=True, stop=True)
        return ps

    def layer_norm(x_src, g_col, b_col, relu, dest):
        """LN over features of feature-major x_src [D, BN] -> dest [D, BN].
        dest = func((x - m) * rstd * g + b), func = relu|identity."""
        for k, (b, ci, c0, w) in enumerate(CHUNKS):
            col0 = b * N + c0
            pt = ps_tr.tile([P, P], F32, tag="trF")
            nc.tensor.transpose(pt[0:w, 0:D], x_src[:, col0:col0 + w], identF[:])
            nc.scalar.copy(x_nm[0:w, k, :], pt[0:w, 0:D])
            nc.vector.bn_stats(out=stats[0:w, k, :], in_=x_nm[0:w, k, :])
            nc.vector.bn_aggr(out=mv[0:w, k, :], in_=stats[0:w, k, :])
            nc.scalar.activation(out=rstd[0:w, k, :], in_=mv[0:w, k, 1:2],
                                 func=AFT.Sqrt, bias=eps_col[0:w])
            nc.vector.reciprocal(out=rstd[0:w, k, :], in_=rstd[0:w, k, :])
            nc.vector.tensor_scalar(
                xn_nm[0:w, k, :], x_nm[0:w, k, :],
                mv[0:w, k, 0:1], rstd[0:w, k, 0:1], AOP.subtract, AOP.mult)
            pt2 = ps_tr.tile([P, P], F32, tag="trF")
            nc.tensor.transpose(pt2[0:D, 0:w], xn_nm[0:w, k, :],
                                identF[0:w, 0:w])
            nc.scalar.activation(
                out=dest[:, col0:col0 + w], in_=pt2[0:D, 0:w],
                func=(AFT.Relu if relu else AFT.Identity),
                bias=b_col, scale=g_col)

    # ---------------- prologue: input MLP
    for k, (b, ci, c0, w) in enumerate(CHUNKS):
        nc.sync.dma_start(out=xin[0:w, k, :], in_=a["xc"][b, c0:c0 + w, :])
    for k, (b, ci, c0, w) in enumerate(CHUNKS):
        pt = ps_tr.tile([P, P], F32, tag="trF")
        nc.tensor.transpose(pt[0:6, 0:w], xin[0:w, k, :], identF[0:w, 0:w])
        nc.scalar.copy(x_fm[:, b * N + c0:b * N + c0 + w], pt[0:6, 0:w])

    ps1 = mm_big(in1WT[:], x_fm[:, 0:BN])
    nc.scalar.activation(out=x_sb[:, 0:BN], in_=ps1[:, 0:BN],
                         func=AFT.Identity, bias=bcol(0))
    layer_norm(x_sb, bcol(2), bcol(3), True, x_sb)
    ps2 = mm_big(in2WT[:], x_sb[:, 0:BN])
    nc.scalar.activation(out=x_sb[:, 0:BN], in_=ps2[:, 0:BN],
                         func=AFT.Identity, bias=bcol(1))
    layer_norm(x_sb, bcol(4), bcol(5), False, h_fm)

    # ---------------- GAT layers
    for l in range(L):
        wl = WT3[:, (l * 3 + 0) * D:(l * 3 + 0) * D + D]
        wr = WT3[:, (l * 3 + 1) * D:(l * 3 + 1) * D + D]
        pw = WT3[:, (l * 3 + 2) * D:(l * 3 + 2) * D + D]
        att04_l = att04[:, l * H:(l + 1) * H]
        att06_l = att06[:, l * H:(l + 1) * H]
        eet_l = eetT[:, l * 12:(l + 1) * 12]
        sedB_l = sedB[:, l * H:(l + 1) * H]
        sedT_l = sedT[:, l * H:(l + 1) * H]

        # gs, gd (feature-major, f32)
        psg = mm_big(wl, h_fm[:, 0:BN])
        nc.scalar.activation(out=gs_sb[:, 0:BN], in_=psg[:, 0:BN],
                             func=AFT.Identity, bias=bcol(c_blbr + 2 * l))
        psg = mm_big(wr, h_fm[:, 0:BN])
        nc.scalar.activation(out=gd_sb[:, 0:BN], in_=psg[:, 0:BN],
                             func=AFT.Identity, bias=bcol(c_blbr + 2 * l + 1))

        # gs3[q] = gs + eet[3p+q]  (bf16)
        for q in range(NO):
            for b in range(BPC):
                for p, (p0, p1) in enumerate(PBLOCKS):
                    nc.vector.tensor_scalar(
                        gs3[:, q, b, p0:p1], gs_sb[:, b * N + p0:b * N + p1],
                        eet_l[:, 3 * p + q:3 * p + q + 1], None, AOP.add)

        # suT = (0.6*att_blk).T @ gs -> [8, BN]
        ps_su = ps_mm.tile([P, BN], F32, tag="mmbig")
        nc.tensor.matmul(ps_su[0:H, 0:BN], att06_l, gs_sb[:, 0:BN],
                         start=True, stop=True)
        nc.scalar.copy(suT[:, 0:BN], ps_su[0:H, 0:BN])

        # gs_nm: node-major gs (bf16) with interleaved den-ones cols
        for k, (b, ci, c0, w) in enumerate(CHUNKS):
            gsb = work.tile([P, P], BF16, tag="gsb")
            nc.vector.tensor_copy(gsb[:, 0:w], gs_sb[:, b * N + c0:b * N + c0 + w])
            ptb = ps_tr.tile([P, P], BF16, tag="trB")
            nc.tensor.transpose(ptb[0:w, 0:D], gsb[:, 0:w], identB[:])
            dst = gs_nm[0:w, ci, b, :].rearrange("p (h s) -> p h s", s=17)
            nc.scalar.copy(dst[:, :, 0:16],
                           ptb[0:w, 0:D].rearrange("p (h c) -> p h c", c=16))

        # diagonal-category correction scores
        for b in range(BPC):
            nc.vector.tensor_tensor(
                zdiag[:, b, 0:N], gs_sb[:, b * N:b * N + N],
                gd_sb[:, b * N:b * N + N], op=AOP.add)
            for p, (p0, p1) in enumerate(PBLOCKS):
                nc.scalar.activation(
                    out=azdB[:, b, p0:p1], in_=zdiag[:, b, p0:p1],
                    func=AFT.Abs, bias=eet_l[:, 4 * p:4 * p + 1])
                nc.scalar.activation(
                    out=azdT[:, b, p0:p1], in_=zdiag[:, b, p0:p1],
                    func=AFT.Abs, bias=eet_l[:, 9 + p:9 + p + 1])
            for ci, (c0, c1) in enumerate(NCHUNKS):
                w = c1 - c0
                for var, azd, sed in ((0, azdB, sedB_l), (1, azdT, sedT_l)):
                    psd = ps_sm.tile([P, H], F32, tag="psd")
                    nc.tensor.matmul(psd[0:w, :], azd[:, b, c0:c1], att04_l,
                                     start=True, stop=False)
                    nc.tensor.matmul(psd[0:w, :], gs_sb[:, b * N + c0:b * N + c1],
                                     att06_l, start=False, stop=False)
                    nc.tensor.matmul(psd[0:w, :], P1h[:, b * N + c0:b * N + c1],
                                     sed, start=False, stop=True)
                    nc.scalar.activation(
                        out=eBeT[0:w, b, ci, var * H:(var + 1) * H],
                        in_=psd[0:w, :], func=AFT.Exp)
            for ci, (c0, c1) in enumerate(NCHUNKS):
                w = c1 - c0
                nc.vector.tensor_tensor(
                    dE[0:w, b, ci, :], eBeT[0:w, b, ci, H:2 * H],
                    eBeT[0:w, b, ci, 0:H], op=AOP.subtract)

        # block scores per (b, J-group): affine + az-contract, exp-evict
        for b in range(BPC):
            for g, (j0, j1) in enumerate(JGROUPS):
                gw = j1 - j0
                ncols = gw * H
                Sps = [ps_S.tile([P, JG * H], F32, tag=f"S{ic}", name=f"S{ic}")
                       for ic in range(2)]
                for ic, (c0, c1) in enumerate(NCHUNKS):
                    w = c1 - c0
                    nc.tensor.matmul(
                        Sps[ic][0:w, 0:ncols], P1h[:, b * N + c0:b * N + c1],
                        se06f[:, (l * N + j0) * H:(l * N + j1) * H],
                        start=True, stop=False)
                    nc.tensor.matmul(
                        Sps[ic][0:w, 0:ncols], suT[:, b * N + c0:b * N + c1],
                        I8t[:, 0:ncols], start=False, stop=False)
                for jj, J in enumerate(range(j0, j1)):
                    az = work.tile([P, NP], BF16, tag="az")
                    if jj % 3 == 2:
                        # ACT-fused |gs3 + gd| (offload from DVE)
                        nc.scalar.activation(
                            out=az[:], in_=gs3[:, POS[J], b, :], func=AFT.Abs,
                            bias=gd_sb[:, b * N + J:b * N + J + 1])
                    else:
                        zt = work.tile([P, NP], BF16, tag="zt")
                        nc.vector.tensor_scalar(
                            zt[:], gs3[:, POS[J], b, :],
                            gd_sb[:, b * N + J:b * N + J + 1], None, AOP.add)
                        nc.vector.tensor_scalar(
                            az[:].bitcast(mybir.dt.uint16),
                            zt[:].bitcast(mybir.dt.uint16),
                            0x7FFF, None, AOP.bitwise_and)
                    for ic, (c0, c1) in enumerate(NCHUNKS):
                        w = c1 - c0
                        nc.tensor.matmul(
                            Sps[ic][0:w, jj * H:(jj + 1) * H],
                            az[:, c0:c1], att04_l,
                            start=False, stop=(jj == gw - 1))
                for ic, (c0, c1) in enumerate(NCHUNKS):
                    w = c1 - c0
                    src = Sps[ic][0:w, 0:ncols].rearrange("p (j h) -> p j h", h=H)
                    dst = E_im[0:w, ic, b, :, j0:j1].rearrange("p h j -> p j h")
                    nc.scalar.activation(out=dst, in_=src, func=AFT.Exp)

        # aggregation: [num|den](J, b, h) = sum_i E * [gs|1]
        for jc, (jc0, jc1) in enumerate(NCHUNKS):
            wj = jc1 - jc0
            Ops = ps_O.tile([P, BPC * H * 17], F32, tag="Ops")
            OpsR = Ops[:].rearrange("p (b h s) -> p b h s", b=BPC, s=17)
            for b in range(BPC):
                for h in range(H):
                    for ic, (c0, c1) in enumerate(NCHUNKS):
                        w = c1 - c0
                        nc.tensor.matmul(
                            Ops[0:wj, (b * H + h) * 17:(b * H + h + 1) * 17],
                            E_im[0:w, ic, b, h, jc0:jc1],
                            gs_nm[0:w, ic, b, h * 17:(h + 1) * 17],
                            start=(ic == 0), stop=(ic == 1))
            for b in range(BPC):
                nc.vector.tensor_tensor(
                    OpsR[0:wj, b, :, 16], OpsR[0:wj, b, :, 16],
                    dE[0:wj, b, jc, :], op=AOP.add)
                gsd = gs_nm[0:wj, jc, b, :].rearrange("p (h s) -> p h s", s=17)
                nc.vector.tensor_tensor(
                    ntmp[0:wj, :].rearrange("p (h c) -> p h c", c=16),
                    gsd[:, :, 0:16],
                    _bcast_inner(dE[0:wj, b, jc, :], 16), op=AOP.mult)
                nc.vector.tensor_tensor(
                    OpsR[0:wj, b, :, 0:16], OpsR[0:wj, b, :, 0:16],
                    ntmp[0:wj, :].rearrange("p (h c) -> p h c", c=16),
                    op=AOP.add)
            nc.vector.reciprocal(
                out=rden[0:wj, jc, :],
                in_=OpsR[0:wj, :, :, 16].rearrange("p b h -> p (b h)"))
            for b in range(BPC):
                nc.vector.tensor_tensor(
                    o_nm[0:wj, jc, b, :].rearrange("p (h c) -> p h c", c=16),
                    OpsR[0:wj, b, :, 0:16],
                    _bcast_inner(rden[0:wj, jc, b * H:(b + 1) * H], 16),
                    op=AOP.mult)

        # projection + LN + relu + residual
        for k, (b, ci, c0, w) in enumerate(CHUNKS):
            pt = ps_tr.tile([P, P], F32, tag="trF")
            nc.tensor.transpose(pt[0:D, 0:w], o_nm[0:w, ci, b, :],
                                identF[0:w, 0:w])
            nc.scalar.copy(ofm[:, b * N + c0:b * N + c0 + w], pt[0:D, 0:w])
        psx = mm_big(pw, ofm[:, 0:BN])
        nc.scalar.activation(out=x_sb[:, 0:BN], in_=psx[:, 0:BN],
                             func=AFT.Identity, bias=bcol(c_pbf + l))
        layer_norm(x_sb, bcol(c_lngb + 2 * l), bcol(c_lngb + 2 * l + 1),
                   True, tmp_fm)
        nc.vector.tensor_tensor(h_fm[:, 0:BN], h_fm[:, 0:BN],
                                tmp_fm[:, 0:BN], op=AOP.add)

    # ---------------- output head
    pso = ps_mm.tile([P, BN], F32, tag="mmbig")
    nc.tensor.matmul(pso[0:2, 0:BN], oWT[:], h_fm[:, 0:BN], start=True, stop=True)
    nc.scalar.activation(out=out_fm[:, 0:BN], in_=pso[0:2, 0:BN],
                         func=AFT.Identity, bias=ob2[:])
    for k, (b, ci, c0, w) in enumerate(CHUNKS):
        pt = ps_tr.tile([P, P], F32, tag="trF")
        nc.tensor.transpose(pt[0:w, 0:2], out_fm[:, b * N + c0:b * N + c0 + w],
                            identF[0:2, 0:2])
        nc.scalar.copy(out_nm[0:w, k, :], pt[0:w, 0:2])
        nc.sync.dma_start(out=out_ap[b, c0:c0 + w, :], in_=out_nm[0:w, k, :])


# ---------------------------------------------------------------- host side
def _host_consts(inputs):
    import ml_dtypes

    f32 = np.float32

    def bf(x):
        return np.ascontiguousarray(np.asarray(x, f32).astype(ml_dtypes.bfloat16))

    emb = np.asarray(inputs["emb"], f32)
    We = np.asarray(inputs["We"], f32)
    att = np.asarray(inputs["att"], f32)
    Wl = np.asarray(inputs["Wl"], f32)
    Wr = np.asarray(inputs["Wr"], f32)
    pW = np.asarray(inputs["pW"], f32)

    pos = np.asarray(POS)
    eet = np.einsum("kd,lod->lko", emb, We)                     # [L,12,D]
    eetT = np.ascontiguousarray(np.transpose(eet, (0, 2, 1)))   # [L,D,12]
    se = np.einsum("lkhc,lhc->lkh", eet.reshape(L, 12, H, C), att)

    att_blk = np.zeros((L, D, H), f32)
    for h in range(H):
        att_blk[:, h * C:(h + 1) * C, h] = att[:, h, :]

    se06f = np.zeros((L, NO, N * H), f32)
    for p in range(NO):
        for j in range(N):
            se06f[:, p, j * H:(j + 1) * H] = 0.6 * se[:, p * NO + pos[j], :]
    sedB = 0.6 * np.stack([se[:, p * NO + p, :] for p in range(NO)], 1)
    sedT = 0.6 * np.stack([se[:, NO * NO + p, :] for p in range(NO)], 1)

    I8t = np.zeros((H, JG * H), f32)
    for j in range(JG):
        for h in range(H):
            I8t[h, j * H + h] = 1.0
    P1h = np.zeros((NO, BN), f32)
    for b in range(BPC):
        for i in range(N):
            P1h[pos[i], b * N + i] = 1.0

    WT3 = np.stack([np.stack([Wl[l].T, Wr[l].T, pW[l].T], 0)
                    for l in range(L)], 0)
    pbf = np.asarray(inputs["pb"], f32) + np.einsum(
        "lod,ld->lo", pW, np.asarray(inputs["cb"], f32))

    return {
        "in1WT": np.ascontiguousarray(np.asarray(inputs["in1_W"], f32).T),
        "in2WT": np.ascontiguousarray(np.asarray(inputs["in2_W"], f32).T),
        "WT3": np.ascontiguousarray(WT3),
        "oWT": np.ascontiguousarray(np.asarray(inputs["oW"], f32).T),
        "eetT": eetT,
        "att04": bf(0.4 * att_blk),
        "att06": np.ascontiguousarray(0.6 * att_blk),
        "se06f": bf(se06f),
        "sedB": bf(sedB),
        "sedT": bf(sedT),
        "I8t": bf(I8t),
        "P1h": bf(P1h),
        "bias2": np.ascontiguousarray(np.stack(
            [np.asarray(inputs["in1_b"], f32), np.asarray(inputs["in2_b"], f32)], 0)),
        "ln12": np.ascontiguousarray(np.stack(
            [np.asarray(inputs["ln1_g"], f32), np.asarray(inputs["ln1_b"], f32),
             np.asarray(inputs["ln2_g"], f32), np.asarray(inputs["ln2_b"], f32)], 0)),
        "blbr": np.ascontiguousarray(np.stack(
            [np.stack([np.asarray(inputs["bl"], f32)[l],
                       np.asarray(inputs["br"], f32)[l]], 0) for l in range(L)], 0)),
        "pbf": np.ascontiguousarray(pbf),
        "lngb": np.ascontiguousarray(np.stack(
            [np.stack([np.asarray(inputs["lng"], f32)[l],
                       np.asarray(inputs["lnb"], f32)[l]], 0) for l in range(L)], 0)),
        "ob2": np.ascontiguousarray(np.asarray(inputs["ob"], f32).reshape(2, 1)),
    }


_NC_CACHE = None


def get_nc():
    global _NC_CACHE
    if _NC_CACHE is None:
        _NC_CACHE = build_nc()
    return _NC_CACHE


def make_in_maps(inputs):
    consts = _host_consts(inputs)
    x = np.asarray(inputs["x"], np.float32)
    in_maps = []
    for core in range(NCORES):
        m = dict(consts)
        m["xc"] = np.ascontiguousarray(x[core * BPC:(core + 1) * BPC])
        in_maps.append(m)
    return in_maps


def kernel(**inputs):
    nc = get_nc()
    in_maps = make_in_maps(inputs)
    res = run_bass_kernel_spmd(nc, in_maps, list(range(NCORES)))
    out = np.concatenate([res.results[i]["out"] for i in range(NCORES)], axis=0)
    return out.astype(np.float32)


# revision 15
# speedup vs baseline: 6.9831x; 6.9831x over previous
"""Trainium2 Bass kernel for nn_GATv2Model (gnn_message_passing).

Sharding: data-parallel over batch (B=16 -> 2 graphs x 8 NeuronCores), all
weights/constants replicated; per-core compute fully SBUF-resident.

Math (reference):
  h = LN2(relu(LN1(x@in1W.T))@in2W.T)
  per layer: gs = h@Wl.T+bl; gd = h@Wr.T+br       [B,N,H,C]
    z[b,i,j] = gs[b,i] + gd[b,j] + ee[cat[i,j]]
    a = att.leaky_relu(z) = 0.6*(su_i+sv_j+se_cat) + sum_c 0.4*att*|z|
    A = softmax_i(a); o = A.gs; h += relu(LN(o@pW.T + cb-fold))
Factorizations used:
  * cat[i,j] = 3p(i)+q(j) off-diagonal (12 categories): ee enters via
    gs3_q = gs + eet[3p+q] (9 cheap per-orbit adds, not N^2 work).
  * softmax normalizer cancels per-(b,j,h) constants => sv term dropped.
  * unnormalized exp: E = exp(a'), o = (E.[gs|1]) num/den; scores are O(1)
    so no max-subtraction needed.
  * diagonal category (cat[i,i]=9+p): computed separately per node for both
    categories; dE = exp(true)-exp(block) added to num/den.
Score inner loop per (b,J): az[hc,i] = |gs3_q(J) + gd[:,J]| as ONE fused DVE
tensor_scalar (op0=add, op1=abs_max) in bf16; PE contracts az [128,m<=128]
against block-diag 0.4*att [128,8] accumulating S_psum[i,(J,h)]; linear terms
enter PSUM via k=3 (orbit one-hot) and k=8 (head-delta) matmuls; ACT applies
exp while evicting PSUM -> E[i,(h,J)] bf16 which is exactly the layout the
aggregation matmuls (contract over i) consume. No transposes in the loop.
"""

import numpy as np
from contextlib import ExitStack

import concourse.bass as bass
import concourse.tile as tile
from concourse import mybir
from concourse.bass_utils import run_bass_kernel_spmd
from concourse.bass_interp import get_hw_module
from concourse.masks import make_identity

# ---------------------------------------------------------------- constants
N, D, H, C, L, B = 207, 128, 8, 16, 4, 16
NO, SZ, EPS = 3, 69, 1e-5
NCORES = 8
BPC = B // NCORES          # graphs per core
BN = BPC * N               # 414
NP = 208                   # padded N (even free dims)
JG = 64                    # J-group: 64*8 = 512 PSUM f32 cols = 1 bank
F32 = mybir.dt.float32
BF16 = mybir.dt.bfloat16
AOP = mybir.AluOpType
AFT = mybir.ActivationFunctionType

POS = (np.arange(N) // SZ).tolist()
NCHUNKS = [(0, 128), (128, N)]
PBLOCKS = [(p * SZ, min((p + 1) * SZ, N)) for p in range(NO)]
JGROUPS = [(g * JG, min((g + 1) * JG, N)) for g in range((N + JG - 1) // JG)]
CHUNKS = [(b, ci, c0, c1 - c0) for b in range(BPC)
          for ci, (c0, c1) in enumerate(NCHUNKS)]

MAX_WAITS = 1  # this walrus build rejects >1 sync wait per instruction

# dev knobs (ablation / tuning). ACT_MOD: every ACT_MOD-th az goes to ACT.
ACT_MOD = 3
ABLATE = set()  # {"az_mm", "az_dve", "az_act", "affine", "agg"}


def _split_excess_waits(nc):
    """Hoist excess per-instruction sync waits onto preceding NoOps."""
    n_fixed = 0
    for f in nc.m.functions:
        for blk in f.blocks:
            if not any(
                i.sync_info and len(i.sync_info.on_wait) > MAX_WAITS
                for i in blk.instructions
            ):
                continue
            new_insts = []
            for inst in blk.instructions:
                si = inst.sync_info
                if si is not None and len(si.on_wait) > MAX_WAITS:
                    waits = list(si.on_wait)
                    extra, keep = waits[:-MAX_WAITS], waits[-MAX_WAITS:]
                    k = 0
                    while extra:
                        chunk, extra = extra[:MAX_WAITS], extra[MAX_WAITS:]
                        new_insts.append(
                            mybir.InstNoOp(
                                name=f"{inst.name}-w{k}",
                                engine=inst.engine,
                                sync_info=mybir.SyncInfo(on_wait=chunk, on_update=[]),
                            )
                        )
                        k += 1
                    inst.sync_info = mybir.SyncInfo(
                        on_wait=keep, on_update=list(si.on_update)
                    )
                    n_fixed += 1
                new_insts.append(inst)
            blk.instructions = new_insts
    return n_fixed


def _bcast_inner(ap_2d, reps):
    """[P, n] AP -> [P, n, reps] AP with step-0 innermost broadcast."""
    dims = list(ap_2d.ap)
    assert len(dims) == 2, dims
    return bass.AP(tensor=ap_2d.tensor, offset=ap_2d.offset,
                   ap=[dims[0], dims[1], [0, reps]])


# ---------------------------------------------------------------- builder
def build_nc(hw=True):
    nc = bass.Bass("TRN2", target_bir_lowering=False, debug=False)

    def din(name, shape, dt=F32):
        return nc.dram_tensor(name, shape, dt, kind="ExternalInput").ap()

    a = {
        "xc": din("xc", [BPC, N, 6]),
        "in1WT": din("in1WT", [6, D]),
        "in2WT": din("in2WT", [D, D]),
        "WT3": din("WT3", [L, 3, D, D]),            # Wl.T, Wr.T, pW.T
        "oWT": din("oWT", [D, 2]),
        "eetT": din("eetT", [L, D, 12]),            # [l, hc, cat]
        "att04": din("att04", [L, D, H], BF16),     # 0.4*att block-diag
        "att06": din("att06", [L, D, H]),           # 0.6*att block-diag
        "se06f": din("se06f", [L, NO, N * H], BF16),
        "sedB": din("sedB", [L, NO, H], BF16),
        "sedT": din("sedT", [L, NO, H], BF16),
        "I8t": din("I8t", [H, JG * H], BF16),
        "P1h": din("P1h", [NO, BN], BF16),
        "bias2": din("bias2", [2, D]),              # in1_b, in2_b
        "ln12": din("ln12", [4, D]),                # g1,b1,g2,b2
        "blbr": din("blbr", [L, 2, D]),
        "pbf": din("pbf", [L, D]),                  # pb + pW@cb
        "lngb": din("lngb", [L, 2, D]),
        "ob2": din("ob2", [2, 1]),
    }
    out_ap = nc.dram_tensor("out", [BPC, N, 2], F32, kind="ExternalOutput").ap()

    with tile.TileContext(nc) as tc:
        with ExitStack() as ctx:
            _body(ctx, tc, a, out_ap)

    if hw:
        nc.m = get_hw_module(nc.m)
        _split_excess_waits(nc)
    return nc


def _body(ctx, tc, a, out_ap):
    nc = tc.nc
    P = 128
    const = ctx.enter_context(tc.tile_pool(name="const", bufs=1))
    state = ctx.enter_context(tc.tile_pool(name="state", bufs=1))
    work = ctx.enter_context(tc.tile_pool(name="work", bufs=4))
    # PSUM: 8 banks total. S0+S1 (1+1) + mmbig (1) + Ops (1) + trF (1) +
    # trB (1) + psd (1) = 7.
    ps_S = ctx.enter_context(tc.tile_pool(name="ps_S", bufs=1, space="PSUM"))
    ps_O = ctx.enter_context(tc.tile_pool(name="ps_O", bufs=1, space="PSUM"))
    ps_mm = ctx.enter_context(tc.tile_pool(name="ps_mm", bufs=1, space="PSUM"))
    ps_tr = ctx.enter_context(tc.tile_pool(name="ps_tr", bufs=1, space="PSUM"))
    ps_sm = ctx.enter_context(tc.tile_pool(name="ps_sm", bufs=1, space="PSUM"))

    # ---------------- constants into SBUF
    def load(name, shape, dt=F32, perm=None, split=None):
        """DMA a[name] (optionally dim-permuted) into a [P0, prod(rest)] tile."""
        t = const.tile(shape, dt, tag=name)
        src = a[name] if perm is None else a[name].rearrange(perm)
        if len(src.shape) > 2:
            names = " ".join(f"d{i}" for i in range(1, len(src.shape)))
            kw = {f"d{i}": src.shape[i] for i in range(1, len(src.shape))}
            dst = t[:].rearrange(f"p ({names}) -> p {names}", **kw)
        else:
            dst = t[:]
        nc.sync.dma_start(out=dst, in_=src)
        return t

    in1WT = load("in1WT", [6, D])
    in2WT = load("in2WT", [D, D])
    WT3 = load("WT3", [D, L * 3 * D], perm="l t i o -> i l t o",
               split="i (l t o) -> i l t o")
    oWT = load("oWT", [D, 2])
    eetT = load("eetT", [D, L * 12], perm="l i c -> i l c",
                split="i (l c) -> i l c")
    att04 = load("att04", [D, L * H], BF16, perm="l i h -> i l h",
                 split="i (l h) -> i l h")
    att06 = load("att06", [D, L * H], perm="l i h -> i l h",
                 split="i (l h) -> i l h")
    se06f = load("se06f", [NO, L * N * H], BF16, perm="l p x -> p l x",
                 split="p (l x) -> p l x")
    sedB = load("sedB", [NO, L * H], BF16, perm="l p h -> p l h",
                split="p (l h) -> p l h")
    sedT = load("sedT", [NO, L * H], BF16, perm="l p h -> p l h",
                split="p (l h) -> p l h")
    I8t = load("I8t", [H, JG * H], BF16)
    P1h = load("P1h", [NO, BN], BF16)
    ob2 = load("ob2", [2, 1])

    # per-partition bias/gain columns, indexed by feature hc (=partition in FM)
    # cols: [in1_b, in2_b, g1, b1, g2, b2, (bl,br)*L, pbf*L, (lng,lnb)*L]
    NBC = 6 + 2 * L + L + 2 * L
    bias_cols = const.tile([D, NBC], F32, tag="bias_cols")
    nc.sync.dma_start(out=bias_cols[:, 0:2], in_=a["bias2"].rearrange("t d -> d t"))
    nc.sync.dma_start(out=bias_cols[:, 2:6], in_=a["ln12"].rearrange("t d -> d t"))
    c_blbr = 6
    nc.sync.dma_start(out=bias_cols[:, c_blbr:c_blbr + 2 * L],
                      in_=a["blbr"].rearrange("l t d -> d (l t)"))
    c_pbf = c_blbr + 2 * L
    nc.sync.dma_start(out=bias_cols[:, c_pbf:c_pbf + L],
                      in_=a["pbf"].rearrange("l d -> d l"))
    c_lngb = c_pbf + L
    nc.sync.dma_start(out=bias_cols[:, c_lngb:c_lngb + 2 * L],
                      in_=a["lngb"].rearrange("l t d -> d (l t)"))

    def bcol(i):
        return bias_cols[:, i:i + 1]

    eps_col = const.tile([P, 1], F32, tag="eps_col")
    nc.vector.memset(eps_col[:], EPS)
    identF = const.tile([P, P], F32, tag="identF")
    make_identity(nc, identF[:])
    identB = const.tile([P, P], BF16, tag="identB")
    nc.vector.tensor_copy(identB[:], identF[:])

    # ---------------- persistent state tiles
    h_fm = state.tile([P, BN], F32, tag="h_fm")
    gs_sb = state.tile([P, BN], F32, tag="gs_sb")
    gd_sb = state.tile([P, BN], F32, tag="gd_sb")
    gs3 = state.tile([P, NO, BPC, NP], BF16, tag="gs3")
    suT = state.tile([H, BN], BF16, tag="suT")
    gs_nm = state.tile([P, 2, BPC, H * 17], BF16, tag="gs_nm")
    E_im = state.tile([P, 2, BPC, H, NP], BF16, tag="E_im")
    dE = state.tile([P, BPC, 2, H], F32, tag="dE")
    o_nm = state.tile([P, 2, BPC, D], F32, tag="o_nm")
    x_sb = state.tile([P, BN], F32, tag="x_sb")
    ofm = state.tile([P, BN], F32, tag="ofm")
    tmp_fm = state.tile([P, BN], F32, tag="tmp_fm")
    x_nm = state.tile([P, 4, D], F32, tag="x_nm")
    xn_nm = state.tile([P, 4, D], F32, tag="xn_nm")
    stats = state.tile([P, 4, 6], F32, tag="stats")
    mv = state.tile([P, 4, 2], F32, tag="mv")
    rstd = state.tile([P, 4, 1], F32, tag="rstd")
    rden = state.tile([P, 2, BPC * H], F32, tag="rden")
    zdiag = state.tile([P, BPC, NP], F32, tag="zdiag")
    azdB = state.tile([P, BPC, NP], BF16, tag="azdB")
    azdT = state.tile([P, BPC, NP], BF16, tag="azdT")
    eBeT = state.tile([P, BPC, 2, 2 * H], F32, tag="eBeT")
    ntmp = state.tile([P, D], F32, tag="ntmp")
    xin = state.tile([P, 4, 6], F32, tag="xin")
    x_fm = state.tile([6, BN], F32, tag="x_fm")
    out_fm = state.tile([2, BN], F32, tag="out_fm")
    out_nm = state.tile([P, 4, 2], F32, tag="out_nm")

    nc.vector.memset(gs3[:], 0.0)
    nc.vector.memset(gs_nm[:], 0.0)
    for ci in range(2):
        for b in range(BPC):
            nc.vector.memset(gs_nm[:, ci, b, 16::17], 1.0)  # den "ones" cols

    # ---------------- helpers
    def mm_big(lhsT, rhs):
        ps = ps_mm.tile([P, BN], F32, tag="mmbig")
        nc.tensor.matmul(ps[:, 0:BN], lhsT, rhs, start=True, stop=True)
        return ps

    def layer_norm(x_src, g_col, b_col, relu, dest):
        """LN over features of feature-major x_src [D, BN] -> dest [D, BN].
        dest = func((x - m) * rstd * g + b), func = relu|identity."""
        for k, (b, ci, c0, w) in enumerate(CHUNKS):
            col0 = b * N + c0
            pt = ps_tr.tile([P, P], F32, tag="trF")
            nc.tensor.transpose(pt[0:w, 0:D], x_src[:, col0:col0 + w], identF[:])
            nc.scalar.copy(x_nm[0:w, k, :], pt[0:w, 0:D])
            nc.vector.bn_stats(out=stats[0:w, k, :], in_=x_nm[0:w, k, :])
            nc.vector.bn_aggr(out=mv[0:w, k, :], in_=stats[0:w, k, :])
            nc.scalar.activation(out=rstd[0:w, k, :], in_=mv[0:w, k, 1:2],
                                 func=AFT.Sqrt, bias=eps_col[0:w])
            nc.vector.reciprocal(out=rstd[0:w, k, :], in_=rstd[0:w, k, :])
            nc.vector.tensor_scalar(
                xn_nm[0:w, k, :], x_nm[0:w, k, :],
                mv[0:w, k, 0:1], rstd[0:w, k, 0:1], AOP.subtract, AOP.mult)
            pt2 = ps_tr.tile([P, P], F32, tag="trF")
            nc.tensor.transpose(pt2[0:D, 0:w], xn_nm[0:w, k, :],
                                identF[0:w, 0:w])
            nc.scalar.activation(
                out=dest[:, col0:col0 + w], in_=pt2[0:D, 0:w],
                func=(AFT.Relu if relu else AFT.Identity),
                bias=b_col, scale=g_col)

    # ---------------- prologue: input MLP
    for k, (b, ci, c0, w) in enumerate(CHUNKS):
        nc.sync.dma_start(out=xin[0:w, k, :], in_=a["xc"][b, c0:c0 + w, :])
    for k, (b, ci, c0, w) in enumerate(CHUNKS):
        pt = ps_tr.tile([P, P], F32, tag="trF")
        nc.tensor.transpose(pt[0:6, 0:w], xin[0:w, k, :], identF[0:w, 0:w])
        nc.scalar.copy(x_fm[:, b * N + c0:b * N + c0 + w], pt[0:6, 0:w])

    ps1 = mm_big(in1WT[:], x_fm[:, 0:BN])
    nc.scalar.activation(out=x_sb[:, 0:BN], in_=ps1[:, 0:BN],
                         func=AFT.Identity, bias=bcol(0))
    layer_norm(x_sb, bcol(2), bcol(3), True, x_sb)
    ps2 = mm_big(in2WT[:], x_sb[:, 0:BN])
    nc.scalar.activation(out=x_sb[:, 0:BN], in_=ps2[:, 0:BN],
                         func=AFT.Identity, bias=bcol(1))
    layer_norm(x_sb, bcol(4), bcol(5), False, h_fm)

    # ---------------- GAT layers
    for l in range(L):
        wl = WT3[:, (l * 3 + 0) * D:(l * 3 + 0) * D + D]
        wr = WT3[:, (l * 3 + 1) * D:(l * 3 + 1) * D + D]
        pw = WT3[:, (l * 3 + 2) * D:(l * 3 + 2) * D + D]
        att04_l = att04[:, l * H:(l + 1) * H]
        att06_l = att06[:, l * H:(l + 1) * H]
        eet_l = eetT[:, l * 12:(l + 1) * 12]
        sedB_l = sedB[:, l * H:(l + 1) * H]
        sedT_l = sedT[:, l * H:(l + 1) * H]

        # gs, gd (feature-major, f32)
        psg = mm_big(wl, h_fm[:, 0:BN])
        nc.scalar.activation(out=gs_sb[:, 0:BN], in_=psg[:, 0:BN],
                             func=AFT.Identity, bias=bcol(c_blbr + 2 * l))
        psg = mm_big(wr, h_fm[:, 0:BN])
        nc.scalar.activation(out=gd_sb[:, 0:BN], in_=psg[:, 0:BN],
                             func=AFT.Identity, bias=bcol(c_blbr + 2 * l + 1))

        # gs3[q] = gs + eet[3p+q]  (bf16)
        for q in range(NO):
            for b in range(BPC):
                for p, (p0, p1) in enumerate(PBLOCKS):
                    nc.vector.tensor_scalar(
                        gs3[:, q, b, p0:p1], gs_sb[:, b * N + p0:b * N + p1],
                        eet_l[:, 3 * p + q:3 * p + q + 1], None, AOP.add)

        # suT = (0.6*att_blk).T @ gs -> [8, BN]
        ps_su = ps_mm.tile([P, BN], F32, tag="mmbig")
        nc.tensor.matmul(ps_su[0:H, 0:BN], att06_l, gs_sb[:, 0:BN],
                         start=True, stop=True)
        nc.scalar.copy(suT[:, 0:BN], ps_su[0:H, 0:BN])

        # gs_nm: node-major gs (bf16) with interleaved den-ones cols
        for k, (b, ci, c0, w) in enumerate(CHUNKS):
            gsb = work.tile([P, P], BF16, tag="gsb")
            nc.vector.tensor_copy(gsb[:, 0:w], gs_sb[:, b * N + c0:b * N + c0 + w])
            ptb = ps_tr.tile([P, P], BF16, tag="trB")
            nc.tensor.transpose(ptb[0:w, 0:D], gsb[:, 0:w], identB[:])
            dst = gs_nm[0:w, ci, b, :].rearrange("p (h s) -> p h s", s=17)
            nc.scalar.copy(dst[:, :, 0:16],
                           ptb[0:w, 0:D].rearrange("p (h c) -> p h c", c=16))

        # diagonal-category correction scores
        for b in range(BPC) if "diag" not in ABLATE else []:
            nc.vector.tensor_tensor(
                zdiag[:, b, 0:N], gs_sb[:, b * N:b * N + N],
                gd_sb[:, b * N:b * N + N], op=AOP.add)
            for p, (p0, p1) in enumerate(PBLOCKS):
                nc.scalar.activation(
                    out=azdB[:, b, p0:p1], in_=zdiag[:, b, p0:p1],
                    func=AFT.Abs, bias=eet_l[:, 4 * p:4 * p + 1])
                nc.scalar.activation(
                    out=azdT[:, b, p0:p1], in_=zdiag[:, b, p0:p1],
                    func=AFT.Abs, bias=eet_l[:, 9 + p:9 + p + 1])
            for ci, (c0, c1) in enumerate(NCHUNKS):
                w = c1 - c0
                for var, azd, sed in ((0, azdB, sedB_l), (1, azdT, sedT_l)):
                    psd = ps_sm.tile([P, H], F32, tag="psd")
                    nc.tensor.matmul(psd[0:w, :], azd[:, b, c0:c1], att04_l,
                                     start=True, stop=False)
                    nc.tensor.matmul(psd[0:w, :], gs_sb[:, b * N + c0:b * N + c1],
                                     att06_l, start=False, stop=False)
                    nc.tensor.matmul(psd[0:w, :], P1h[:, b * N + c0:b * N + c1],
                                     sed, start=False, stop=True)
                    nc.scalar.activation(
                        out=eBeT[0:w, b, ci, var * H:(var + 1) * H],
                        in_=psd[0:w, :], func=AFT.Exp)
            for ci, (c0, c1) in enumerate(NCHUNKS):
                w = c1 - c0
                nc.vector.tensor_tensor(
                    dE[0:w, b, ci, :], eBeT[0:w, b, ci, H:2 * H],
                    eBeT[0:w, b, ci, 0:H], op=AOP.subtract)

        # block scores per (b, J-group): affine + az-contract, exp-evict
        for b in range(BPC):
            for g, (j0, j1) in enumerate(JGROUPS):
                gw = j1 - j0
                ncols = gw * H
                Sps = [ps_S.tile([P, JG * H], F32, tag=f"S{ic}", name=f"S{ic}")
                       for ic in range(2)]
                for ic, (c0, c1) in enumerate(NCHUNKS):
                    w = c1 - c0
                    if "affine" in ABLATE:
                        continue
                    nc.tensor.matmul(
                        Sps[ic][0:w, 0:ncols], P1h[:, b * N + c0:b * N + c1],
                        se06f[:, (l * N + j0) * H:(l * N + j1) * H],
                        start=True, stop=False)
                    nc.tensor.matmul(
                        Sps[ic][0:w, 0:ncols], suT[:, b * N + c0:b * N + c1],
                        I8t[:, 0:ncols], start=False, stop=False)
                for jj, J in enumerate(range(j0, j1)):
                    az = work.tile([P, NP], BF16, tag="az")
                    use_act = (ACT_MOD > 0) and (jj % ACT_MOD == ACT_MOD - 1)
                    if use_act and "az_act" not in ABLATE:
                        # ACT-fused |gs3 + gd| (offload from DVE)
                        nc.scalar.activation(
                            out=az[:], in_=gs3[:, POS[J], b, :], func=AFT.Abs,
                            bias=gd_sb[:, b * N + J:b * N + J + 1])
                    elif not use_act and "az_dve" not in ABLATE:
                        zt = work.tile([P, NP], BF16, tag="zt")
                        nc.vector.tensor_scalar(
                            zt[:], gs3[:, POS[J], b, :],
                            gd_sb[:, b * N + J:b * N + J + 1], None, AOP.add)
                        nc.vector.tensor_scalar(
                            az[:].bitcast(mybir.dt.uint16),
                            zt[:].bitcast(mybir.dt.uint16),
                            0x7FFF, None, AOP.bitwise_and)
                    if "az_mm" in ABLATE:
                        continue
                    for ic, (c0, c1) in enumerate(NCHUNKS):
                        w = c1 - c0
                        nc.tensor.matmul(
                            Sps[ic][0:w, jj * H:(jj + 1) * H],
                            az[:, c0:c1], att04_l,
                            start=False, stop=(jj == gw - 1))
                for ic, (c0, c1) in enumerate(NCHUNKS):
                    w = c1 - c0
                    if "exp" in ABLATE:
                        continue
                    src = Sps[ic][0:w, 0:ncols].rearrange("p (j h) -> p j h", h=H)
                    dst = E_im[0:w, ic, b, :, j0:j1].rearrange("p h j -> p j h")
                    nc.scalar.activation(out=dst, in_=src, func=AFT.Exp)

        # aggregation: [num|den](J, b, h) = sum_i E * [gs|1]
        for jc, (jc0, jc1) in enumerate(NCHUNKS) if "agg" not in ABLATE else []:
            wj = jc1 - jc0
            Ops = ps_O.tile([P, BPC * H * 17], F32, tag="Ops")
            OpsR = Ops[:].rearrange("p (b h s) -> p b h s", b=BPC, s=17)
            for b in range(BPC):
                for h in range(H):
                    for ic, (c0, c1) in enumerate(NCHUNKS):
                        w = c1 - c0
                        nc.tensor.matmul(
                            Ops[0:wj, (b * H + h) * 17:(b * H + h + 1) * 17],
                            E_im[0:w, ic, b, h, jc0:jc1],
                            gs_nm[0:w, ic, b, h * 17:(h + 1) * 17],
                            start=(ic == 0), stop=(ic == 1))
            for b in range(BPC):
                nc.vector.tensor_tensor(
                    OpsR[0:wj, b, :, 16], OpsR[0:wj, b, :, 16],
                    dE[0:wj, b, jc, :], op=AOP.add)
                gsd = gs_nm[0:wj, jc, b, :].rearrange("p (h s) -> p h s", s=17)
                nc.vector.tensor_tensor(
                    ntmp[0:wj, :].rearrange("p (h c) -> p h c", c=16),
                    gsd[:, :, 0:16],
                    _bcast_inner(dE[0:wj, b, jc, :], 16), op=AOP.mult)
                nc.vector.tensor_tensor(
                    OpsR[0:wj, b, :, 0:16], OpsR[0:wj, b, :, 0:16],
                    ntmp[0:wj, :].rearrange("p (h c) -> p h c", c=16),
                    op=AOP.add)
            nc.vector.reciprocal(
                out=rden[0:wj, jc, :],
                in_=OpsR[0:wj, :, :, 16].rearrange("p b h -> p (b h)"))
            for b in range(BPC):
                nc.vector.tensor_tensor(
                    o_nm[0:wj, jc, b, :].rearrange("p (h c) -> p h c", c=16),
                    OpsR[0:wj, b, :, 0:16],
                    _bcast_inner(rden[0:wj, jc, b * H:(b + 1) * H], 16),
                    op=AOP.mult)

        # projection + LN + relu + residual
        for k, (b, ci, c0, w) in enumerate(CHUNKS):
            pt = ps_tr.tile([P, P], F32, tag="trF")
            nc.tensor.transpose(pt[0:D, 0:w], o_nm[0:w, ci, b, :],
                                identF[0:w, 0:w])
            nc.scalar.copy(ofm[:, b * N + c0:b * N + c0 + w], pt[0:D, 0:w])
        psx = mm_big(pw, ofm[:, 0:BN])
        nc.scalar.activation(out=x_sb[:, 0:BN], in_=psx[:, 0:BN],
                             func=AFT.Identity, bias=bcol(c_pbf + l))
        layer_norm(x_sb, bcol(c_lngb + 2 * l), bcol(c_lngb + 2 * l + 1),
                   True, tmp_fm)
        nc.vector.tensor_tensor(h_fm[:, 0:BN], h_fm[:, 0:BN],
                                tmp_fm[:, 0:BN], op=AOP.add)

    # ---------------- output head
    pso = ps_mm.tile([P, BN], F32, tag="mmbig")
    nc.tensor.matmul(pso[0:2, 0:BN], oWT[:], h_fm[:, 0:BN], start=True, stop=True)
    nc.scalar.activation(out=out_fm[:, 0:BN], in_=pso[0:2, 0:BN],
                         func=AFT.Identity, bias=ob2[:])
    for k, (b, ci, c0, w) in enumerate(CHUNKS):
        pt = ps_tr.tile([P, P], F32, tag="trF")
        nc.tensor.transpose(pt[0:w, 0:2], out_fm[:, b * N + c0:b * N + c0 + w],
                            identF[0:2, 0:2])
        nc.scalar.copy(out_nm[0:w, k, :], pt[0:w, 0:2])
        nc.sync.dma_start(out=out_ap[b, c0:c0 + w, :], in_=out_nm[0:w, k, :])


# ---------------------------------------------------------------- host side
def _host_consts(inputs):
    import ml_dtypes

    f32 = np.float32

    def bf(x):
        return np.ascontiguousarray(np.asarray(x, f32).astype(ml_dtypes.bfloat16))

    emb = np.asarray(inputs["emb"], f32)
    We = np.asarray(inputs["We"], f32)
    att = np.asarray(inputs["att"], f32)
    Wl = np.asarray(inputs["Wl"], f32)
    Wr = np.asarray(inputs["Wr"], f32)
    pW = np.asarray(inputs["pW"], f32)

    pos = np.asarray(POS)
    eet = np.einsum("kd,lod->lko", emb, We)                     # [L,12,D]
    eetT = np.ascontiguousarray(np.transpose(eet, (0, 2, 1)))   # [L,D,12]
    se = np.einsum("lkhc,lhc->lkh", eet.reshape(L, 12, H, C), att)

    att_blk = np.zeros((L, D, H), f32)
    for h in range(H):
        att_blk[:, h * C:(h + 1) * C, h] = att[:, h, :]

    se06f = np.zeros((L, NO, N * H), f32)
    for p in range(NO):
        for j in range(N):
            se06f[:, p, j * H:(j + 1) * H] = 0.6 * se[:, p * NO + pos[j], :]
    sedB = 0.6 * np.stack([se[:, p * NO + p, :] for p in range(NO)], 1)
    sedT = 0.6 * np.stack([se[:, NO * NO + p, :] for p in range(NO)], 1)

    I8t = np.zeros((H, JG * H), f32)
    for j in range(JG):
        for h in range(H):
            I8t[h, j * H + h] = 1.0
    P1h = np.zeros((NO, BN), f32)
    for b in range(BPC):
        for i in range(N):
            P1h[pos[i], b * N + i] = 1.0

    WT3 = np.stack([np.stack([Wl[l].T, Wr[l].T, pW[l].T], 0)
                    for l in range(L)], 0)
    pbf = np.asarray(inputs["pb"], f32) + np.einsum(
        "lod,ld->lo", pW, np.asarray(inputs["cb"], f32))

    return {
        "in1WT": np.ascontiguousarray(np.asarray(inputs["in1_W"], f32).T),
        "in2WT": np.ascontiguousarray(np.asarray(inputs["in2_W"], f32).T),
        "WT3": np.ascontiguousarray(WT3),
        "oWT": np.ascontiguousarray(np.asarray(inputs["oW"], f32).T),
        "eetT": eetT,
        "att04": bf(0.4 * att_blk),
        "att06": np.ascontiguousarray(0.6 * att_blk),
        "se06f": bf(se06f),
        "sedB": bf(sedB),
        "sedT": bf(sedT),
        "I8t": bf(I8t),
        "P1h": bf(P1h),
        "bias2": np.ascontiguousarray(np.stack(
            [np.asarray(inputs["in1_b"], f32), np.asarray(inputs["in2_b"], f32)], 0)),
        "ln12": np.ascontiguousarray(np.stack(
            [np.asarray(inputs["ln1_g"], f32), np.asarray(inputs["ln1_b"], f32),
             np.asarray(inputs["ln2_g"], f32), np.asarray(inputs["ln2_b"], f32)], 0)),
        "blbr": np.ascontiguousarray(np.stack(
            [np.stack([np.asarray(inputs["bl"], f32)[l],
                       np.asarray(inputs["br"], f32)[l]], 0) for l in range(L)], 0)),
        "pbf": np.ascontiguousarray(pbf),
        "lngb": np.ascontiguousarray(np.stack(
            [np.stack([np.asarray(inputs["lng"], f32)[l],
                       np.asarray(inputs["lnb"], f32)[l]], 0) for l in range(L)], 0)),
        "ob2": np.ascontiguousarray(np.asarray(inputs["ob"], f32).reshape(2, 1)),
    }


_NC_CACHE = None


def get_nc():
    global _NC_CACHE
    if _NC_CACHE is None:
        _NC_CACHE = build_nc()
    return _NC_CACHE


def make_in_maps(inputs):
    consts = _host_consts(inputs)
    x = np.asarray(inputs["x"], np.float32)
    in_maps = []
    for core in range(NCORES):
        m = dict(consts)
        m["xc"] = np.ascontiguousarray(x[core * BPC:(core + 1) * BPC])
        in_maps.append(m)
    return in_maps


def kernel(**inputs):
    nc = get_nc()
    in_maps = make_in_maps(inputs)
    res = run_bass_kernel_spmd(nc, in_maps, list(range(NCORES)))
    out = np.concatenate([res.results[i]["out"] for i in range(NCORES)], axis=0)
    return out.astype(np.float32)
